# revision 31
# baseline (speedup 1.0000x reference)
"""Trainium2 Bass kernel for nn_Attend (segmented linear-attention + causal softmax blend).

Self-contained: hardcodes shapes b=2,h=8,n=8192,d=64, SEGMENT_LEN=1024, 8 cores.
Sharding: batch*heads (16 pairs) -> 2 pairs per core.
"""
import sys

sys.path.insert(0, "/opt/trn_rl_repo")

import numpy as np
import ml_dtypes

import concourse.bass as bass
import concourse.bacc as bacc
import concourse.tile as tile
from concourse import mybir
from concourse.bass_utils import run_bass_kernel_spmd

F32 = mybir.dt.float32
BF16 = mybir.dt.bfloat16
AX = mybir.AxisListType
OP = mybir.AluOpType
ACT = mybir.ActivationFunctionType

import os
KSTAGE = int(os.environ.get("KSTAGE", "4"))
B, H, N, D = 2, 8, 8192, 64
L = 1024          # segment length
S = N // L        # 8 segments
NB = 128          # block rows
NT = L // NB      # 8 tiles per segment
PBH = 2           # bh pairs per core
NCORES = 8


def build_nc():
    nc = bacc.Bacc(None, target_bir_lowering=False)
    # per-core inputs
    q_d = nc.dram_tensor("q", [PBH, N, D], F32, kind="ExternalInput")
    k_d = nc.dram_tensor("k", [PBH, N, D], F32, kind="ExternalInput")
    v_d = nc.dram_tensor("v", [PBH, N, D], F32, kind="ExternalInput")
    gate_d = nc.dram_tensor("gate", [1, PBH], F32, kind="ExternalInput")
    cos_d = nc.dram_tensor("cos_t", [N, D], BF16, kind="ExternalInput")
    sinf_d = nc.dram_tensor("sinf_t", [N, D], BF16, kind="ExternalInput")
    mask_d = nc.dram_tensor("mask_t", [NB, NB], BF16, kind="ExternalInput")
    ident_d = nc.dram_tensor("ident_t", [NB, NB], F32, kind="ExternalInput")
    out_d = nc.dram_tensor("out", [PBH, N, D], F32, kind="ExternalOutput")

    with tile.TileContext(nc) as tc:
        with (
            tc.tile_pool(name="const", bufs=1) as constp,
            tc.tile_pool(name="tbl", bufs=1) as tblp,
            tc.tile_pool(name="ld", bufs=2) as ldp,
            tc.tile_pool(name="rot", bufs=2) as rotp,
            tc.tile_pool(name="tp", bufs=2) as tpp,
            tc.tile_pool(name="pt", bufs=8) as ptp,
            tc.tile_pool(name="epi", bufs=2) as epip,
            tc.tile_pool(name="outp", bufs=2) as outp,
            tc.tile_pool(name="psS", bufs=3, space="PSUM") as psS,
            tc.tile_pool(name="psO", bufs=1, space="PSUM") as psO,
            tc.tile_pool(name="psN", bufs=1, space="PSUM") as psN,
            tc.tile_pool(name="psA", bufs=1, space="PSUM") as psA,
            tc.tile_pool(name="psE", bufs=2, space="PSUM") as psE,
        ):
            # ---- constants loaded once ----
            cos_sb = tblp.tile([NB, N // NB, D], BF16, tag="cos")
            sinf_sb = tblp.tile([NB, N // NB, D], BF16, tag="sinf")
            nc.sync.dma_start(out=cos_sb, in_=cos_d.rearrange("(g p) d -> p g d", p=NB))
            nc.sync.dma_start(out=sinf_sb, in_=sinf_d.rearrange("(g p) d -> p g d", p=NB))
            mask_sb = constp.tile([NB, NB], BF16, tag="mask")
            nc.sync.dma_start(out=mask_sb, in_=mask_d[:, :])
            ident_sb = constp.tile([NB, NB], F32, tag="ident")
            nc.sync.dma_start(out=ident_sb, in_=ident_d[:, :])
            identb_sb = constp.tile([NB, NB], BF16, tag="identb")
            nc.vector.tensor_copy(out=identb_sb, in_=ident_sb)
            # gate (DRAM partition-broadcast) -> sigmoid
            gate_b = constp.tile([NB, PBH], F32, tag="gate")
            gate_ap = gate_d[:, :]
            bcast = bass.AP(tensor=gate_ap.tensor, offset=gate_ap.offset,
                            ap=[[0, NB], [1, PBH]])
            nc.sync.dma_start(out=gate_b, in_=bcast)
            gs_b = constp.tile([NB, PBH], F32, tag="gsb")
            nc.scalar.activation(out=gs_b, in_=gate_b, func=ACT.Sigmoid)
            omgs_b = constp.tile([NB, PBH], F32, tag="omgs")
            nc.vector.tensor_scalar(out=omgs_b, in0=gs_b, scalar1=-1.0, scalar2=1.0,
                                    op0=OP.mult, op1=OP.add)

            for bh in range(PBH):
                A_sbuf = constp.tile([D, D + 1], F32, tag=f"A_state{bh}")
                nc.vector.memset(A_sbuf, 0.0)
                for s in range(S):
                    g0 = s * NT  # global block index base of this segment
                    # ---- loads ----
                    q_nat = ldp.tile([NB, NT, D], F32, tag="q_nat")
                    k_nat = ldp.tile([NB, NT, D], F32, tag="k_nat")
                    v_nat = ldp.tile([NB, NT, D], F32, tag="v_nat")
                    src = lambda t: t[bh, s * L:(s + 1) * L, :].rearrange(
                        "(g p) d -> p g d", p=NB)
                    nc.sync.dma_start(out=q_nat, in_=src(q_d))
                    nc.sync.dma_start(out=k_nat, in_=src(k_d))
                    nc.sync.dma_start(out=v_nat, in_=src(v_d))

                    # ---- v_aug (bf16 + ones col) on gpsimd ----
                    v_aug = rotp.tile([NB, NT, D + 1], BF16, tag="v_aug")
                    nc.gpsimd.tensor_copy(out=v_aug[:, :, 0:D], in_=v_nat)
                    nc.gpsimd.memset(v_aug[:, :, D:D + 1], 1.0)

                    # ---- rotary (DVE) ----
                    cs = cos_sb[:, g0:g0 + NT, :]
                    sf = sinf_sb[:, g0:g0 + NT, :]
                    hD = D // 2

                    def rotary(x_nat, tag):
                        u = rotp.tile([NB, NT, D], BF16, tag=f"u_{tag}")
                        nc.vector.tensor_mul(u[:, :, 0:hD], x_nat[:, :, hD:D], sf[:, :, 0:hD])
                        nc.vector.tensor_mul(u[:, :, hD:D], x_nat[:, :, 0:hD], sf[:, :, hD:D])
                        xc = rotp.tile([NB, NT, D], BF16, tag=f"xc_{tag}")
                        nc.vector.tensor_mul(xc, x_nat, cs)
                        xr = rotp.tile([NB, NT, D], BF16, tag=f"xr_{tag}")
                        nc.vector.tensor_add(xr, xc, u)
                        return xr

                    q_rot = rotary(q_nat, "q")
                    k_rot = rotary(k_nat, "k")

                    # ---- elu+1 = min(exp(x), relu(x)+1) ----
                    def elu1(x_rot, tag):
                        ex = rotp.tile([NB, NT, D], BF16, tag=f"ex_{tag}")
                        nc.scalar.activation(out=ex, in_=x_rot, func=ACT.Exp)
                        rp = rotp.tile([NB, NT, D], BF16, tag=f"rp_{tag}")
                        nc.vector.tensor_scalar(out=rp, in0=x_rot, scalar1=0.0,
                                                scalar2=1.0, op0=OP.max, op1=OP.add)
                        xe = rotp.tile([NB, NT, D], BF16, tag=f"xe_{tag}")
                        nc.vector.tensor_tensor(out=xe, in0=ex, in1=rp, op=OP.min)
                        return xe

                    qe = elu1(q_rot, "q")
                    ke = elu1(k_rot, "k")

                    if KSTAGE == 1:
                        o_nat = outp.tile([NB, NT, D], F32, tag="o_nat")
                        nc.vector.tensor_copy(out=o_nat, in_=qe)
                        nc.sync.dma_start(
                            out=out_d[bh, s * L:(s + 1) * L, :].rearrange(
                                "(g p) d -> p g d", p=NB),
                            in_=o_nat)
                        continue
                    # ---- DMA transposes (SBUF->SBUF, bf16, [128,128] block pairs) ----
                    # slots 0-3: pairs (2u,2u+1) -> top half = even block 2u,
                    #            bottom = odd 2u+1
                    # slots 4-6: pairs (2u+1,2u+2) -> top = odd block 2u+1
                    # slot 7: top = block 7 (dma copy from slot 3 bottom)
                    qTs = tpp.tile([NB, NT, NB], BF16, tag="qTs")
                    kTs = tpp.tile([NB, NT, NB], BF16, tag="kTs")
                    qes = tpp.tile([NB, NT, NB], BF16, tag="qes")
                    for src_t, dst in ((q_rot, qTs), (k_rot, kTs), (qe, qes)):
                        for u in range(4):
                            nc.scalar.dma_start_transpose(
                                out=dst[:, u, :], in_=src_t[:, 2 * u:2 * u + 2, :])
                        for u in range(3):
                            nc.scalar.dma_start_transpose(
                                out=dst[:, 4 + u, :],
                                in_=src_t[:, 2 * u + 1:2 * u + 3, :])
                        nc.sync.dma_start(out=dst[0:D, 7, :], in_=dst[D:NB, 3, :])
                    def top(stack, j):  # [64, 128] view of block j in top half
                        slot = j // 2 if j % 2 == 0 else (7 if j == 7 else 4 + (j - 1) // 2)
                        return stack[0:D, slot, :]

                    if KSTAGE == 2:
                        o_nat = outp.tile([NB, NT, D], F32, tag="o_nat")
                        nc.vector.tensor_copy(out=o_nat[:, 0:4, :],
                                              in_=qTs[:, 0:2, :].rearrange("p a b -> p (a b)").rearrange("p (a b) -> p a b", a=4))
                        nc.vector.tensor_copy(out=o_nat[:, 4:8, :],
                                              in_=kTs[:, 0:2, :].rearrange("p a b -> p (a b)").rearrange("p (a b) -> p a b", a=4))
                        nc.sync.dma_start(
                            out=out_d[bh, s * L:(s + 1) * L, :].rearrange(
                                "(g p) d -> p g d", p=NB),
                            in_=o_nat)
                        continue
                    # ---- linear-attention retrieval (A from previous segments) ----
                    if s > 0:
                        A_sb = epip.tile([D, D + 1], BF16, tag="A_sb")
                        nc.vector.tensor_copy(out=A_sb, in_=A_sbuf)

                    # ---- A update: A += ke_blk^T @ v_aug_blk (PSUM delta) ----
                    A_ps = psA.tile([D, D + 1], F32, tag="A")
                    for t in range(NT):
                        nc.tensor.matmul(A_ps, ke[:, t, :], v_aug[:, t, :],
                                         start=(t == 0), stop=(t == NT - 1))
                    if s < S - 1:
                        nc.vector.tensor_add(A_sbuf, A_sbuf, A_ps)

                    # ---- scores + exp per k-tile ----
                    pts = []
                    for j in range(NT):
                        ptj = ptp.tile([NB, L], BF16, tag="pt")
                        pts.append(ptj)
                        qlo = j * NB
                        # chunk q-tiles into groups of <=4 (512 psum cols)
                        for clo in range(qlo, L, 512):
                            w = min(512, L - clo)
                            st = psS.tile([NB, 512], F32, tag="st")
                            for ii in range(w // NB):
                                i = (clo + ii * NB) // NB
                                nc.tensor.matmul(st[:, ii * NB:(ii + 1) * NB],
                                                 top(kTs, j), top(qTs, i),
                                                 start=True, stop=True)
                            nc.scalar.activation(out=ptj[:, clo:clo + w], in_=st[:, 0:w],
                                                 func=ACT.Exp, scale=0.125)
                        # diagonal block causal mask
                        nc.gpsimd.tensor_mul(ptj[:, qlo:qlo + NB],
                                             ptj[:, qlo:qlo + NB], mask_sb)

                    if KSTAGE == 3:
                        o_nat = outp.tile([NB, NT, D], F32, tag="o_nat")
                        nc.vector.tensor_copy(out=o_nat[:, 0:4, :],
                                              in_=pts[0][:, 0:256].rearrange("p (a b) -> p a b", a=4))
                        nc.vector.tensor_copy(out=o_nat[:, 4:8, :],
                                              in_=pts[1][:, 0:256].rearrange("p (a b) -> p a b", a=4))
                        nc.sync.dma_start(
                            out=out_d[bh, s * L:(s + 1) * L, :].rearrange(
                                "(g p) d -> p g d", p=NB),
                            in_=o_nat)
                        continue
                    # ---- per q-half: PV, num, epilogue ----
                    use_num = (s > 0) and KSTAGE >= 4
                    o_nat = outp.tile([NB, NT, D], F32, tag="o_nat")
                    for hf in range(2):
                        tiles = range(hf * 4, hf * 4 + 4)
                        O_ps = psO.tile([D + 1, 512], F32, tag="O")
                        for i in tiles:
                            io = (i - hf * 4) * NB
                            for j in range(i + 1):
                                nc.tensor.matmul(
                                    O_ps[:, io:io + NB], v_aug[:, j, :],
                                    pts[j][:, i * NB:(i + 1) * NB],
                                    start=(j == 0), stop=(j == i))
                        if use_num:
                            N_ps = psN.tile([D + 1, 512], F32, tag="Nm")
                            for i in tiles:
                                io = (i - hf * 4) * NB
                                nc.tensor.matmul(N_ps[:, io:io + NB],
                                                 A_sb, top(qes, i),
                                                 start=True, stop=True)
                        # copies PSUM->SBUF (bf16, padded to 66 partitions)
                        Osb = epip.tile([D + 2, 512], BF16, tag="Osb")
                        nc.vector.tensor_copy(out=Osb[0:D + 1, :], in_=O_ps)
                        if use_num:
                            Nsb = epip.tile([D + 2, 512], BF16, tag="Nsb")
                            nc.vector.tensor_copy(out=Nsb[0:D + 1, :], in_=N_ps)
                        if KSTAGE == 37:
                            nc.vector.tensor_copy(
                                out=o_nat[0:D, hf * 4:hf * 4 + 4, :],
                                in_=Osb[0:D, 0:256].rearrange(
                                    "p (a b) -> p a b", a=4))
                            continue
                        # transposes -> q-major [128, 4, 66] (even K, bf16)
                        Oq = psE.tile([NB, 4, D + 2], BF16, tag="EQ")
                        for ii in range(4):
                            nc.tensor.transpose(Oq[:, ii, :], Osb[:, ii * NB:(ii + 1) * NB],
                                                identb_sb[0:D + 2, 0:D + 2])
                        if use_num:
                            Nq = psE.tile([NB, 4, D + 2], BF16, tag="EQ")
                            for ii in range(4):
                                nc.tensor.transpose(Nq[:, ii, :],
                                                    Nsb[:, ii * NB:(ii + 1) * NB],
                                                    identb_sb[0:D + 2, 0:D + 2])
                        # blend (DVE, per-tile tensor_scalar with [128,1] scalars)
                        dsm = epip.tile([NB, 4], F32, tag="dsm")
                        nc.vector.tensor_copy(out=dsm, in_=Oq[:, :, D])
                        r2 = epip.tile([NB, 4], F32, tag="r2")
                        nc.vector.reciprocal(out=r2, in_=dsm)
                        r2g = epip.tile([NB, 4], F32, tag="r2g")
                        nc.vector.tensor_scalar_mul(r2g, r2, omgs_b[:, bh:bh + 1])
                        ohalf = o_nat[:, hf * 4:hf * 4 + 4, :]
                        if use_num:
                            r1 = epip.tile([NB, 4], F32, tag="r1")
                            dm = epip.tile([NB, 4], F32, tag="dm")
                            nc.vector.tensor_scalar_add(dm, Nq[:, :, D], 1e-6)  # psum read via ts
                            nc.vector.reciprocal(out=r1, in_=dm)
                            r1g = epip.tile([NB, 4], F32, tag="r1g")
                            nc.vector.tensor_scalar_mul(r1g, r1, gs_b[:, bh:bh + 1])
                            t2 = epip.tile([NB, 4, D], F32, tag="t2")
                            for ii in range(4):
                                nc.vector.tensor_scalar_mul(
                                    t2[:, ii, :], Oq[:, ii, 0:D], r2g[:, ii:ii + 1])
                            for ii in range(4):
                                nc.vector.tensor_scalar_mul(
                                    ohalf[:, ii, :], Nq[:, ii, 0:D], r1g[:, ii:ii + 1])
                            nc.vector.tensor_add(ohalf, ohalf, t2)
                        else:
                            for ii in range(4):
                                nc.vector.tensor_scalar_mul(
                                    ohalf[:, ii, :], Oq[:, ii, 0:D], r2g[:, ii:ii + 1])
                    # ---- store ----
                    nc.sync.dma_start(
                        out=out_d[bh, s * L:(s + 1) * L, :].rearrange(
                            "(g p) d -> p g d", p=NB),
                        in_=o_nat)
    return nc


_NC_CACHE = {}
TRACE = False
LAST_EXEC_NS = None


def _tables():
    inv_freq = (1.0 / (10000.0 ** (np.arange(0, D, 2, dtype=np.float32) / np.float32(D)))).astype(np.float32)
    t = np.arange(N, dtype=np.float32)
    freqs = np.outer(t, inv_freq).astype(np.float32)
    emb = np.concatenate([freqs, freqs], axis=-1)
    cos = np.cos(emb).astype(ml_dtypes.bfloat16)
    sin = np.sin(emb).astype(np.float32)
    sinf = np.concatenate([-sin[:, :32], sin[:, 32:]], axis=-1).astype(ml_dtypes.bfloat16)
    mask = (np.arange(NB)[:, None] <= np.arange(NB)[None, :]).astype(ml_dtypes.bfloat16)
    ident = np.eye(NB, dtype=np.float32)
    return cos, sinf, mask, ident


def kernel(q, k, v, gate):
    q = np.asarray(q, dtype=np.float32)
    k = np.asarray(k, dtype=np.float32)
    v = np.asarray(v, dtype=np.float32)
    gate = np.asarray(gate, dtype=np.float32)
    if "nc" not in _NC_CACHE:
        nc = build_nc()
        nc.finalize()
        _NC_CACHE["nc"] = nc
    nc = _NC_CACHE["nc"]
    cos, sinf, mask, ident = _tables()

    qf = q.reshape(B * H, N, D)
    kf = k.reshape(B * H, N, D)
    vf = v.reshape(B * H, N, D)
    gf = np.broadcast_to(gate.reshape(1, H), (B, H)).reshape(B * H)

    in_maps = []
    for c in range(NCORES):
        sl = slice(c * PBH, (c + 1) * PBH)
        in_maps.append({
            "q": np.ascontiguousarray(qf[sl]),
            "k": np.ascontiguousarray(kf[sl]),
            "v": np.ascontiguousarray(vf[sl]),
            "gate": np.ascontiguousarray(gf[sl]).reshape(1, PBH),
            "cos_t": cos, "sinf_t": sinf, "mask_t": mask, "ident_t": ident,
        })
    global LAST_EXEC_NS
    res = run_bass_kernel_spmd(nc, in_maps, core_ids=list(range(NCORES)),
                               trace=TRACE)
    LAST_EXEC_NS = res.exec_time_ns
    outs = [r["out"] for r in res.results]
    out = np.stack(outs, axis=0).reshape(B, H, N, D)
    return out
